# revision 15
# baseline (speedup 1.0000x reference)
"""Trainium2 Bass kernel for causal multi-head attention.

Problem: B=2, S=2048, D=1024, H=16 heads (DH=64), causal, fp32 reference.

Sharding over 8 NeuronCores: core c handles batch b = c//4 and head group
g = c%4 (4 heads each).  Wq/Wk/Wv are split column-wise (by output head),
Wo row-wise; per-core partial outputs are summed on the host (row-parallel
Wo => partial sums), then bo is added.

Per-core device kernel (all matmul operands bf16, fp32 PSUM accumulation):
  qT/kT = W @ xT            (64, 2048) per head, head pairs stacked on 128 parts
  v     = x @ WvT           (2048, 4*65) with a ones column per head (row sums)
  S^T   = k q^T             scores transposed: (s_k, s_q) tiles
  P     = exp(0.125 * S^T)  ScalarE, fused scale; causal mask applied via
                            multiplicative 0/1 masks on diagonal tiles (DVE)
  ctx^T, sums = [v | 1]^T P accumulated over k tiles in PSUM (M=65)
  ctx^T *= 1/sums           DVE reciprocal + GpSimd partition broadcast
  out   = ctx @ WoT         (2048, 1024) fp32 partial
"""

import sys

sys.path.insert(0, "/opt/trn_rl_repo")

import numpy as np
import ml_dtypes

import concourse.bass as bass
import concourse.bacc as bacc
import concourse.mybir as mybir
import concourse.tile as tile
from concourse.bass_utils import run_bass_kernel_spmd

BF16 = mybir.dt.bfloat16
F32 = mybir.dt.float32
AF = mybir.ActivationFunctionType

B, S, D, H = 2, 2048, 1024, 16
DH = D // H            # 64
NCORES = 8
NH = 4                 # heads per core
DL = NH * DH           # 256 local head dims per core
KD = D // 128          # 8 contraction chunks for projections
SQ = S // 512          # 4 q tiles of 512
ST = S // 128          # 16 s tiles of 128
SCALE = DH ** -0.5     # 0.125

# Module-level caches (build/compile once per process)
_NC = None

# When True, run_bass_kernel_spmd is invoked with trace=True; the last
# BassKernelResults is stored in LAST_RESULTS for test harness inspection.
TRACE = False
LAST_RESULTS = None


def _build_nc():
    nc = bacc.Bacc("TRN2", target_bir_lowering=False, debug=False,
                   num_devices=NCORES)

    xT_d = nc.dram_tensor("xT", [D, S], BF16, kind="ExternalInput")
    wq_d = nc.dram_tensor("wqT", [D, DL], BF16, kind="ExternalInput")
    wk_d = nc.dram_tensor("wkT", [D, DL], BF16, kind="ExternalInput")
    wv_d = nc.dram_tensor("wvT", [D, DL], BF16, kind="ExternalInput")
    wo_d = nc.dram_tensor("woT", [DL, D], BF16, kind="ExternalInput")
    # causal masks for the 4 diagonal offsets, duplicated for the two
    # head halves of a [128, 1024] exp tile
    mask_d = nc.dram_tensor("mask", [128, 4, 1024], BF16, kind="ExternalInput")
    out_d = nc.dram_tensor("out", [S, D], F32, kind="ExternalOutput")

    with tile.TileContext(nc) as tc:
        with (
            tc.tile_pool(name="consts", bufs=1) as consts,
            tc.tile_pool(name="xpool", bufs=1) as xpool,
            tc.tile_pool(name="qkpool", bufs=1) as qkpool,
            tc.tile_pool(name="vpool", bufs=1) as vpool,
            tc.tile_pool(name="cpool", bufs=1) as cpool,
            tc.tile_pool(name="exppool", bufs=3) as exppool,
            tc.tile_pool(name="smallpool", bufs=3) as smallpool,
            tc.tile_pool(name="outpool", bufs=4) as outpool,
            tc.tile_pool(name="spsum", bufs=2, space="PSUM") as spsum,
            tc.tile_pool(name="ctxpsum", bufs=4, space="PSUM") as ctxpsum,
        ):
            # ---- constants ----
            wq_sb = consts.tile([128, KD, DL], BF16)
            wk_sb = consts.tile([128, KD, DL], BF16)
            wv_sb = consts.tile([128, KD, DL], BF16)
            wo_sb = consts.tile([128, 2, D], BF16)
            mask_sb = consts.tile([128, 4, 1024], BF16)
            zb = consts.tile([128, 1], F32)

            nc.sync.dma_start(wq_sb[:], wq_d.rearrange("(kd p) j -> p kd j", p=128))
            nc.sync.dma_start(wk_sb[:], wk_d.rearrange("(kd p) j -> p kd j", p=128))
            nc.sync.dma_start(wv_sb[:], wv_d.rearrange("(kd p) j -> p kd j", p=128))
            nc.sync.dma_start(wo_sb[:], wo_d.rearrange("(kc p) o -> p kc o", p=128))
            nc.sync.dma_start(mask_sb[:], mask_d[:])
            nc.vector.memset(zb[:], 0.0)

            # ---- persistent activations ----
            # x^T blocks: [kd][sq] -> (128 d, 512 s)
            xt = [
                [
                    xpool.tile([128, 512], BF16, name=f"xt_{kd}_{sq}",
                               tag=f"xt_{kd}_{sq}")
                    for sq in range(SQ)
                ]
                for kd in range(KD)
            ]
            # q^T / k^T blocks: [m2][sq] -> (128 = 2 heads x 64, 512 s)
            qT = [
                [
                    qkpool.tile([128, 512], BF16, name=f"qT_{m2}_{sq}",
                                tag=f"qT_{m2}_{sq}")
                    for sq in range(SQ)
                ]
                for m2 in range(2)
            ]
            kT = [
                [
                    qkpool.tile([128, 512], BF16, name=f"kT_{m2}_{sq}",
                                tag=f"kT_{m2}_{sq}")
                    for sq in range(SQ)
                ]
                for m2 in range(2)
            ]
            # v blocks with ones column: [st] -> (128 s, 4 heads, 65)
            vt = [
                vpool.tile([128, NH, DH + 1], BF16, name=f"v_{st}", tag=f"v_{st}")
                for st in range(ST)
            ]
            # normalized ctx^T blocks: [kc][sq] -> (128 = 2 heads x 64, 512 s)
            ctxT = [
                [
                    cpool.tile([128, 512], BF16, name=f"ctxT_{kc}_{sq}",
                               tag=f"ctxT_{kc}_{sq}")
                    for sq in range(SQ)
                ]
                for kc in range(2)
            ]

            xT_r = xT_d.rearrange("(kd p) s -> kd p s", p=128)

            # ---- phase B: QKV projections (per 512-wide s tile) ----
            for sq in range(SQ):
                for kd in range(KD):
                    nc.sync.dma_start(
                        xt[kd][sq][:], xT_r[kd, :, sq * 512:(sq + 1) * 512]
                    )
                for m2 in range(2):
                    q_ps = spsum.tile([128, 512], F32, name="q_ps", tag="s")
                    for kd in range(KD):
                        nc.tensor.matmul(
                            q_ps[:],
                            wq_sb[:, kd, m2 * 128:(m2 + 1) * 128],
                            xt[kd][sq][:],
                            start=(kd == 0),
                            stop=(kd == KD - 1),
                        )
                    nc.scalar.copy(qT[m2][sq][:], q_ps[:])
                    k_ps = spsum.tile([128, 512], F32, name="k_ps", tag="s")
                    for kd in range(KD):
                        nc.tensor.matmul(
                            k_ps[:],
                            wk_sb[:, kd, m2 * 128:(m2 + 1) * 128],
                            xt[kd][sq][:],
                            start=(kd == 0),
                            stop=(kd == KD - 1),
                        )
                    nc.scalar.copy(kT[m2][sq][:], k_ps[:])
                for sti in range(4):
                    st = sq * 4 + sti
                    v_ps = spsum.tile([128, DL], F32, name="v_ps", tag="s")
                    for kd in range(KD):
                        nc.tensor.matmul(
                            v_ps[:],
                            xt[kd][sq][:, sti * 128:(sti + 1) * 128],
                            wv_sb[:, kd, :],
                            start=(kd == 0),
                            stop=(kd == KD - 1),
                        )
                    nc.vector.tensor_copy(
                        vt[st][:, :, 0:DH],
                        v_ps[:].rearrange("p (h d) -> p h d", h=NH),
                    )
                    nc.vector.memset(vt[st][:, :, DH:DH + 1], 1.0)

            # ---- phase C: attention + phase D: output projection, per q tile ----
            # Heads are processed in pairs (the two heads sharing a qT/kT
            # partition chunk). Per k tile, the pair's two score matmuls use
            # disjoint 64-row groups of the PE array (concurrent), land in the
            # two banks of one [128, 1024] PSUM tile, and share one exp and
            # one mask instruction.
            def emit_wo(j, sti, ot):
                # one (st, ot) output-projection tile of q tile j
                st = j * 4 + sti
                o_ps = spsum.tile([128, 512], F32, name="o_ps", tag="s")
                for kc in range(2):
                    nc.tensor.matmul(
                        o_ps[:],
                        ctxT[kc][j][:, sti * 128:(sti + 1) * 128],
                        wo_sb[:, kc, ot * 512:(ot + 1) * 512],
                        start=(kc == 0),
                        stop=(kc == 1),
                    )
                ob = outpool.tile([128, 512], F32, name="ob", tag="ob")
                if ot == 0:
                    nc.scalar.copy(ob[:], o_ps[:])
                else:
                    nc.vector.tensor_copy(ob[:], o_ps[:])
                nc.sync.dma_start(
                    out_d[st * 128:(st + 1) * 128, ot * 512:(ot + 1) * 512],
                    ob[:],
                )

            for j in range(SQ):
                nkt = 4 * j + 4  # causal: k tiles 0 .. 4j+3
                # previous q tile's output projection, interleaved into this
                # k loop so its matmuls/copies fill dependency bubbles
                wo_items = [(j - 1, sti, ot) for sti in range(4)
                            for ot in range(2)] if j > 0 else []
                c_ps = [
                    [
                        ctxpsum.tile([128, 512], F32, name=f"c_ps{p}{i2}",
                                     tag="ctx")
                        for i2 in range(2)
                    ]
                    for p in range(2)
                ]
                # Both head pairs advance through the k loop together so each
                # engine always has the other pair's independent work to run
                # while this pair's exp/mask is in flight.
                for kt in range(nkt):
                    if wo_items:
                        emit_wo(*wo_items.pop(0))
                    for p in range(2):
                        s_ps = spsum.tile([128, 1024], F32, name="s_ps", tag="s")
                        for i2 in range(2):
                            hr = i2 * 64
                            nc.tensor.matmul(
                                s_ps[:, i2 * 512:(i2 + 1) * 512],
                                kT[p][kt // 4][hr:hr + 64,
                                               (kt % 4) * 128:(kt % 4 + 1) * 128],
                                qT[p][j][hr:hr + 64, :],
                                start=True,
                                stop=True,
                            )
                        ex = exppool.tile([128, 1024], BF16, name="ex", tag="ex")
                        nc.scalar.activation(
                            ex[:], s_ps[:], AF.Exp, bias=zb[:], scale=SCALE
                        )
                        if kt >= 4 * j:  # diagonal block: causal mask
                            o = kt - 4 * j
                            nc.vector.tensor_mul(ex[:], ex[:], mask_sb[:, o, :])
                        for i2 in range(2):
                            nc.tensor.matmul(
                                c_ps[p][i2][0:DH + 1, :],
                                vt[kt][:, 2 * p + i2, :],
                                ex[:, i2 * 512:(i2 + 1) * 512],
                                start=(kt == 0),
                                stop=(kt == nkt - 1),
                            )
                for p in range(2):
                    for i2 in range(2):
                        hr = i2 * 64
                        # sums row lives at PSUM partition 64; the only legal
                        # route to partition 0 (for the custom-DVE approx
                        # reciprocal and GpSimd broadcast, both base-0-only)
                        # is equal-base copy + SBUF->SBUF DMA partition move.
                        s_sb = smallpool.tile([65, 512], F32, name="s_sb",
                                              tag="s_sb")
                        nc.vector.tensor_copy(
                            s_sb[64:65, :], c_ps[p][i2][DH:DH + 1, :]
                        )
                        s0 = smallpool.tile([1, 512], F32, name="s0", tag="s0")
                        # SWDGE (gpsimd) queue: tiny bounce must not sit
                        # behind bulk loads/stores in the HWDGE queues
                        nc.gpsimd.dma_start(s0[:], s_sb[64:65, :])
                        inv0 = smallpool.tile([1, 512], F32, name="inv0",
                                              tag="inv0")
                        nc.vector.reciprocal_approx_fast(inv0[:], s0[:])
                        invb = smallpool.tile([64, 512], F32, name="invb",
                                              tag="invb")
                        nc.gpsimd.partition_broadcast(invb[:], inv0[:])
                        nc.vector.tensor_mul(
                            ctxT[p][j][hr:hr + 64, :], c_ps[p][i2][0:DH, :],
                            invb[:],
                        )

            # last q tile's output projection
            for sti in range(4):
                for ot in range(2):
                    emit_wo(SQ - 1, sti, ot)

    nc.compile()
    return nc


def _get_nc():
    global _NC
    if _NC is None:
        _NC = _build_nc()
    return _NC


def _bf16(a):
    return np.ascontiguousarray(a).astype(ml_dtypes.bfloat16)


def kernel(x, Wq, Wk, Wv, Wo, bo):
    global LAST_RESULTS
    x = np.asarray(x, dtype=np.float32)
    Wq = np.asarray(Wq, dtype=np.float32)
    Wk = np.asarray(Wk, dtype=np.float32)
    Wv = np.asarray(Wv, dtype=np.float32)
    Wo = np.asarray(Wo, dtype=np.float32)
    bo = np.asarray(bo, dtype=np.float32)

    # host-side prep
    xT = [_bf16(x[b].T) for b in range(B)]          # (D, S)
    WqT = np.ascontiguousarray(Wq.T)                # (D, D): col slice = head rows
    WkT = np.ascontiguousarray(Wk.T)
    WvT = np.ascontiguousarray(Wv.T)
    WoT = np.ascontiguousarray(Wo.T)                # (D, D): row slice = ctx dims

    kk = np.arange(128, dtype=np.int64)[:, None]
    qq = np.arange(512, dtype=np.int64)[None, :]
    mask1 = np.stack(
        [(qq >= 128 * o + kk).astype(np.float32) for o in range(4)], axis=1
    )  # (128, 4, 512)
    mask = np.concatenate([mask1, mask1], axis=2)  # (128, 4, 1024): both heads
    mask = mask.astype(ml_dtypes.bfloat16)

    in_maps = []
    for c in range(NCORES):
        b, g = divmod(c, 4)
        sl = slice(g * DL, (g + 1) * DL)
        in_maps.append(
            {
                "xT": xT[b],
                "wqT": _bf16(WqT[:, sl]),
                "wkT": _bf16(WkT[:, sl]),
                "wvT": _bf16(WvT[:, sl]),
                "woT": _bf16(WoT[sl, :]),
                "mask": mask,
            }
        )

    nc = _get_nc()
    results = run_bass_kernel_spmd(
        nc, in_maps, core_ids=list(range(NCORES)), trace=TRACE
    )
    LAST_RESULTS = results

    out = np.zeros((B, S, D), dtype=np.float32)
    for c in range(NCORES):
        out[c // 4] += results.results[c]["out"]
    out += bo[None, None, :]
    return out


# revision 19
# speedup vs baseline: 1.0480x; 1.0480x over previous
"""Trainium2 Bass kernel for causal multi-head attention.

Problem: B=2, S=2048, D=1024, H=16 heads (DH=64), causal, fp32 reference.

Sharding over 8 NeuronCores: core c handles batch b = c//4 and head group
g = c%4 (4 heads each).  Wq/Wk/Wv are split column-wise (by output head),
Wo row-wise; per-core partial outputs are summed on the host (row-parallel
Wo => partial sums), then bo is added.

Per-core device kernel (all matmul operands bf16, fp32 PSUM accumulation):
  qT/kT = W @ xT            (64, 2048) per head, head pairs stacked on 128 parts
  v     = x @ WvT           (2048, 4*65) with a ones column per head (row sums)
  S^T   = k q^T             scores transposed: (s_k, s_q) tiles
  P     = exp(0.125 * S^T)  ScalarE, fused scale; causal mask applied via
                            multiplicative 0/1 masks on diagonal tiles (DVE)
  ctx^T, sums = [v | 1]^T P accumulated over k tiles in PSUM (M=65)
  ctx^T *= 1/sums           DVE reciprocal + GpSimd partition broadcast
  out   = ctx @ WoT         (2048, 1024) fp32 partial
"""

import sys

sys.path.insert(0, "/opt/trn_rl_repo")

import numpy as np
import ml_dtypes

import concourse.bass as bass
import concourse.bacc as bacc
import concourse.mybir as mybir
import concourse.tile as tile
from concourse.bass_utils import run_bass_kernel_spmd

BF16 = mybir.dt.bfloat16
F32 = mybir.dt.float32
AF = mybir.ActivationFunctionType

B, S, D, H = 2, 2048, 1024, 16
DH = D // H            # 64
NCORES = 8
NH = 4                 # heads per core
DL = NH * DH           # 256 local head dims per core
KD = D // 128          # 8 contraction chunks for projections
SQ = S // 512          # 4 q tiles of 512
ST = S // 128          # 16 s tiles of 128
SCALE = DH ** -0.5     # 0.125

# Module-level caches (build/compile once per process)
_NC = None

# When True, run_bass_kernel_spmd is invoked with trace=True; the last
# BassKernelResults is stored in LAST_RESULTS for test harness inspection.
TRACE = False
LAST_RESULTS = None


def _build_nc():
    nc = bacc.Bacc("TRN2", target_bir_lowering=False, debug=False,
                   num_devices=NCORES)

    xT_d = nc.dram_tensor("xT", [D, S], BF16, kind="ExternalInput")
    wq_d = nc.dram_tensor("wqT", [D, DL], BF16, kind="ExternalInput")
    wk_d = nc.dram_tensor("wkT", [D, DL], BF16, kind="ExternalInput")
    wv_d = nc.dram_tensor("wvT", [D, DL], BF16, kind="ExternalInput")
    wo_d = nc.dram_tensor("woT", [DL, D], BF16, kind="ExternalInput")
    # causal masks for the 4 diagonal offsets, duplicated for the two
    # head halves of a [128, 1024] exp tile
    mask_d = nc.dram_tensor("mask", [128, 4, 1024], BF16, kind="ExternalInput")
    out_d = nc.dram_tensor("out", [S, D], F32, kind="ExternalOutput")

    with tile.TileContext(nc) as tc:
        with (
            tc.tile_pool(name="consts", bufs=1) as consts,
            tc.tile_pool(name="xpool", bufs=1) as xpool,
            tc.tile_pool(name="qkpool", bufs=1) as qkpool,
            tc.tile_pool(name="vpool", bufs=1) as vpool,
            tc.tile_pool(name="cpool", bufs=1) as cpool,
            tc.tile_pool(name="exppool", bufs=3) as exppool,
            tc.tile_pool(name="smallpool", bufs=3) as smallpool,
            tc.tile_pool(name="outpool", bufs=4) as outpool,
            tc.tile_pool(name="spsum", bufs=3, space="PSUM") as spsum,
            tc.tile_pool(name="ctxpsum", bufs=2, space="PSUM") as ctxpsum,
        ):
            # ---- constants ----
            wq_sb = consts.tile([128, KD, DL], BF16)
            wk_sb = consts.tile([128, KD, DL], BF16)
            wv_sb = consts.tile([128, KD, DL], BF16)
            wo_sb = consts.tile([128, 2, D], BF16)
            mask_sb = consts.tile([128, 4, 1024], BF16)
            zb = consts.tile([128, 1], F32)

            nc.sync.dma_start(wq_sb[:], wq_d.rearrange("(kd p) j -> p kd j", p=128))
            nc.sync.dma_start(wk_sb[:], wk_d.rearrange("(kd p) j -> p kd j", p=128))
            nc.sync.dma_start(wv_sb[:], wv_d.rearrange("(kd p) j -> p kd j", p=128))
            nc.sync.dma_start(wo_sb[:], wo_d.rearrange("(kc p) o -> p kc o", p=128))
            nc.sync.dma_start(mask_sb[:], mask_d[:])
            nc.vector.memset(zb[:], 0.0)

            # ---- persistent activations ----
            # x^T blocks: [kd][sq] -> (128 d, 512 s)
            xt = [
                [
                    xpool.tile([128, 512], BF16, name=f"xt_{kd}_{sq}",
                               tag=f"xt_{kd}_{sq}")
                    for sq in range(SQ)
                ]
                for kd in range(KD)
            ]
            # q^T / k^T blocks: [m2][sq] -> (128 = 2 heads x 64, 512 s)
            qT = [
                [
                    qkpool.tile([128, 512], BF16, name=f"qT_{m2}_{sq}",
                                tag=f"qT_{m2}_{sq}")
                    for sq in range(SQ)
                ]
                for m2 in range(2)
            ]
            kT = [
                [
                    qkpool.tile([128, 512], BF16, name=f"kT_{m2}_{sq}",
                                tag=f"kT_{m2}_{sq}")
                    for sq in range(SQ)
                ]
                for m2 in range(2)
            ]
            # v blocks with ones column: [st] -> (128 s, 4 heads, 65)
            vt = [
                vpool.tile([128, NH, DH + 1], BF16, name=f"v_{st}", tag=f"v_{st}")
                for st in range(ST)
            ]
            # normalized ctx^T blocks: [kc][sq] -> (128 = 2 heads x 64, 512 s)
            ctxT = [
                [
                    cpool.tile([128, 512], BF16, name=f"ctxT_{kc}_{sq}",
                               tag=f"ctxT_{kc}_{sq}")
                    for sq in range(SQ)
                ]
                for kc in range(2)
            ]

            xT_r = xT_d.rearrange("(kd p) s -> kd p s", p=128)

            # ---- phase B: QKV projections (per 512-wide s tile) ----
            for sq in range(SQ):
                for kd in range(KD):
                    nc.sync.dma_start(
                        xt[kd][sq][:], xT_r[kd, :, sq * 512:(sq + 1) * 512]
                    )
                for m2 in range(2):
                    q_ps = spsum.tile([128, 512], F32, name="q_ps", tag="s")
                    for kd in range(KD):
                        nc.tensor.matmul(
                            q_ps[:],
                            wq_sb[:, kd, m2 * 128:(m2 + 1) * 128],
                            xt[kd][sq][:],
                            start=(kd == 0),
                            stop=(kd == KD - 1),
                        )
                    nc.scalar.copy(qT[m2][sq][:], q_ps[:])
                    k_ps = spsum.tile([128, 512], F32, name="k_ps", tag="s")
                    for kd in range(KD):
                        nc.tensor.matmul(
                            k_ps[:],
                            wk_sb[:, kd, m2 * 128:(m2 + 1) * 128],
                            xt[kd][sq][:],
                            start=(kd == 0),
                            stop=(kd == KD - 1),
                        )
                    nc.scalar.copy(kT[m2][sq][:], k_ps[:])
                for sti in range(4):
                    st = sq * 4 + sti
                    v_ps = spsum.tile([128, DL], F32, name="v_ps", tag="s")
                    for kd in range(KD):
                        nc.tensor.matmul(
                            v_ps[:],
                            xt[kd][sq][:, sti * 128:(sti + 1) * 128],
                            wv_sb[:, kd, :],
                            start=(kd == 0),
                            stop=(kd == KD - 1),
                        )
                    nc.vector.tensor_copy(
                        vt[st][:, :, 0:DH],
                        v_ps[:].rearrange("p (h d) -> p h d", h=NH),
                    )
                    nc.vector.memset(vt[st][:, :, DH:DH + 1], 1.0)

            # ---- phase C: attention + phase D: output projection, per q tile ----
            # Heads are processed in pairs (the two heads sharing a qT/kT
            # partition chunk). Per k tile, the pair's two score matmuls use
            # disjoint 64-row groups of the PE array (concurrent), land in the
            # two banks of one [128, 1024] PSUM tile, and share one exp and
            # one mask instruction.
            def emit_wo(j, sti, ot):
                # one (st, ot) output-projection tile of q tile j
                st = j * 4 + sti
                o_ps = spsum.tile([128, 512], F32, name="o_ps", tag="s")
                for kc in range(2):
                    nc.tensor.matmul(
                        o_ps[:],
                        ctxT[kc][j][:, sti * 128:(sti + 1) * 128],
                        wo_sb[:, kc, ot * 512:(ot + 1) * 512],
                        start=(kc == 0),
                        stop=(kc == 1),
                    )
                ob = outpool.tile([128, 512], F32, name="ob", tag="ob")
                if ot == 0:
                    nc.scalar.copy(ob[:], o_ps[:])
                else:
                    nc.vector.tensor_copy(ob[:], o_ps[:])
                nc.sync.dma_start(
                    out_d[st * 128:(st + 1) * 128, ot * 512:(ot + 1) * 512],
                    ob[:],
                )

            for j in range(SQ):
                nkt = 4 * j + 4  # causal: k tiles 0 .. 4j+3
                # previous q tile's output projection: burst it between pairs
                wo_items = [(j - 1, sti, ot) for sti in range(4)
                            for ot in range(2)] if j > 0 else []
                for p in range(2):
                    for (wj, wsti, wot) in wo_items[p * 4:(p + 1) * 4]:
                        emit_wo(wj, wsti, wot)
                    c_ps = [
                        ctxpsum.tile([128, 512], F32, name=f"c_ps{i2}",
                                     tag="ctx")
                        for i2 in range(2)
                    ]
                    # Software-pipelined k loop with lag-2 ctx accumulation:
                    # PE runs scores(kt) while ACT exps kt-1 and the ctx
                    # matmuls consume exp(kt-2) — neither engine waits on the
                    # other at matmul granularity.
                    LAG = 2
                    exs = {}

                    def emit_scores(kt):
                        s_ps = spsum.tile([128, 1024], F32, name="s_ps",
                                          tag="s")
                        for i2 in range(2):
                            hr = i2 * 64
                            nc.tensor.matmul(
                                s_ps[:, i2 * 512:(i2 + 1) * 512],
                                kT[p][kt // 4][hr:hr + 64,
                                               (kt % 4) * 128:(kt % 4 + 1) * 128],
                                qT[p][j][hr:hr + 64, :],
                                start=True,
                                stop=True,
                            )
                        ex = exppool.tile([128, 1024], BF16, name="ex",
                                          tag="ex")
                        nc.scalar.activation(
                            ex[:], s_ps[:], AF.Exp, bias=zb[:], scale=SCALE
                        )
                        if kt >= 4 * j:  # diagonal block: causal mask
                            o = kt - 4 * j
                            nc.vector.tensor_mul(ex[:], ex[:], mask_sb[:, o, :])
                        exs[kt] = ex

                    def emit_ctx(kt):
                        ex = exs.pop(kt)
                        for i2 in range(2):
                            nc.tensor.matmul(
                                c_ps[i2][0:DH + 1, :],
                                vt[kt][:, 2 * p + i2, :],
                                ex[:, i2 * 512:(i2 + 1) * 512],
                                start=(kt == 0),
                                stop=(kt == nkt - 1),
                            )

                    for kt in range(nkt):
                        emit_scores(kt)
                        if kt >= LAG:
                            emit_ctx(kt - LAG)
                    for kt in range(nkt - LAG, nkt):
                        emit_ctx(kt)

                    for i2 in range(2):
                        hr = i2 * 64
                        # sums row lives at PSUM partition 64; the only legal
                        # route to partition 0 (for the custom-DVE approx
                        # reciprocal and GpSimd broadcast, both base-0-only)
                        # is equal-base copy + SBUF->SBUF DMA partition move.
                        s_sb = smallpool.tile([65, 512], F32, name="s_sb",
                                              tag="s_sb")
                        nc.vector.tensor_copy(
                            s_sb[64:65, :], c_ps[i2][DH:DH + 1, :]
                        )
                        s0 = smallpool.tile([1, 512], F32, name="s0", tag="s0")
                        # SWDGE (gpsimd) queue: tiny bounce must not sit
                        # behind bulk loads/stores in the HWDGE queues
                        nc.gpsimd.dma_start(s0[:], s_sb[64:65, :])
                        inv0 = smallpool.tile([1, 512], F32, name="inv0",
                                              tag="inv0")
                        nc.vector.reciprocal_approx_fast(inv0[:], s0[:])
                        invb = smallpool.tile([64, 512], F32, name="invb",
                                              tag="invb")
                        nc.gpsimd.partition_broadcast(invb[:], inv0[:])
                        nc.vector.tensor_mul(
                            ctxT[p][j][hr:hr + 64, :], c_ps[i2][0:DH, :],
                            invb[:],
                        )

            # last q tile's output projection
            for sti in range(4):
                for ot in range(2):
                    emit_wo(SQ - 1, sti, ot)

    nc.compile()
    return nc


def _get_nc():
    global _NC
    if _NC is None:
        _NC = _build_nc()
    return _NC


def _bf16(a):
    return np.ascontiguousarray(a).astype(ml_dtypes.bfloat16)


def kernel(x, Wq, Wk, Wv, Wo, bo):
    global LAST_RESULTS
    x = np.asarray(x, dtype=np.float32)
    Wq = np.asarray(Wq, dtype=np.float32)
    Wk = np.asarray(Wk, dtype=np.float32)
    Wv = np.asarray(Wv, dtype=np.float32)
    Wo = np.asarray(Wo, dtype=np.float32)
    bo = np.asarray(bo, dtype=np.float32)

    # host-side prep
    xT = [_bf16(x[b].T) for b in range(B)]          # (D, S)
    WqT = np.ascontiguousarray(Wq.T)                # (D, D): col slice = head rows
    WkT = np.ascontiguousarray(Wk.T)
    WvT = np.ascontiguousarray(Wv.T)
    WoT = np.ascontiguousarray(Wo.T)                # (D, D): row slice = ctx dims

    kk = np.arange(128, dtype=np.int64)[:, None]
    qq = np.arange(512, dtype=np.int64)[None, :]
    mask1 = np.stack(
        [(qq >= 128 * o + kk).astype(np.float32) for o in range(4)], axis=1
    )  # (128, 4, 512)
    mask = np.concatenate([mask1, mask1], axis=2)  # (128, 4, 1024): both heads
    mask = mask.astype(ml_dtypes.bfloat16)

    in_maps = []
    for c in range(NCORES):
        b, g = divmod(c, 4)
        sl = slice(g * DL, (g + 1) * DL)
        in_maps.append(
            {
                "xT": xT[b],
                "wqT": _bf16(WqT[:, sl]),
                "wkT": _bf16(WkT[:, sl]),
                "wvT": _bf16(WvT[:, sl]),
                "woT": _bf16(WoT[sl, :]),
                "mask": mask,
            }
        )

    nc = _get_nc()
    results = run_bass_kernel_spmd(
        nc, in_maps, core_ids=list(range(NCORES)), trace=TRACE
    )
    LAST_RESULTS = results

    out = np.zeros((B, S, D), dtype=np.float32)
    for c in range(NCORES):
        out[c // 4] += results.results[c]["out"]
    out += bo[None, None, :]
    return out
